# revision 11
# baseline (speedup 1.0000x reference)
"""Cross multi-head attention on 8 Trainium2 NeuronCores.

Sharding: batch x head-group. Core c handles batch b = c//4 and heads
4*(c%4) .. 4*(c%4)+3. Wq tensor-sharded by head (columns), Wo by its
input (head) dim (rows); the 4 partial outputs per batch are summed on
the host.

v2 structure (all engine-level changes vs the 354us baseline):
  - all inputs bf16 (halves DMA), x streamed in 512-col blocks so the
    first Q-projection matmul starts after ~1MB of DMA instead of 9MB.
  - scores for a HEAD PAIR run concurrently in the PE array via row
    tiling: heads (2p, 2p+1) keep their K^T and Q^T stacked in
    partitions 0-63 / 64-127 of shared tiles; the two 64-contraction
    matmuls occupy disjoint row groups (tile_position (0,0) / (64,0)).
  - the per-block softmax denominators (row 64 of the attn PSUM, via
    the ones-column of V_ext) are DMA-scattered [1,1024]->[128,8],
    inverted with reciprocal_approx_fast (~100ns vs 6.5us serial DVE
    reciprocal that stalled the PE every block), multiplied by the
    scattered q-mask, and DMA-gathered back for the PE broadcast.
  - output projection + store run per q-block, interleaved with the
    next block's attention.
"""

import numpy as np
import ml_dtypes

import concourse.bass as bass
import concourse.mybir as mybir
import concourse.tile as tile
from concourse.bass_utils import run_bass_kernel_spmd

F32 = mybir.dt.float32
F32R = mybir.dt.float32r
BF16 = mybir.dt.bfloat16
AF = mybir.ActivationFunctionType
BF = ml_dtypes.bfloat16

B, H, NQ, NKV, D, DK = 2, 16, 2048, 2048, 1024, 64
HPC = 4              # heads per core
CPB = 4              # cores per batch
KT_TILES = NKV // 128
MC = D // 128        # model-dim chunks
QB = 1024            # q block width for scores/exp/attn
NQB = NQ // QB
SCALE = 1.0 / 8.0    # 1/sqrt(DK)


def _split_excess_waits(nc, limit=1):
    """This walrus build rejects instructions carrying several sem waits.
    Move excess waits onto standalone EventSemaphore instructions placed
    directly before the offender on the same (FIFO) engine queue."""
    n = 0
    for f in nc.m.functions:
        for bb in f.blocks:
            out = []
            for inst in bb.instructions:
                si = inst.sync_info
                waits = list(si.on_wait) if si is not None else []
                if len(waits) > limit:
                    excess, keep = waits[:-limit], waits[-limit:]
                    for w in excess:
                        n += 1
                        out.append(mybir.InstEventSemaphore(
                            name=f"wsplit-{n}-{inst.name}",
                            engine=inst.engine,
                            ins=[], outs=[],
                            sync_info=mybir.SyncInfo(on_wait=[w], on_update=[]),
                        ))
                    si.on_wait = keep
                out.append(inst)
            bb.instructions = out
    return n


def _build_program():
    nc = bass.Bass("TRN2", target_bir_lowering=False, debug=False, num_devices=8)

    d_wq = nc.declare_dram_parameter("wq", [D, HPC * DK], BF16, isOutput=False)
    d_xt = nc.declare_dram_parameter("xt", [D, NQ], BF16, isOutput=False)
    d_kt = nc.declare_dram_parameter("kt2", [2, 128, NKV], BF16, isOutput=False)
    d_vx = nc.declare_dram_parameter("vext", [HPC, 128, KT_TILES * (DK + 1)], BF16, isOutput=False)
    d_wo = nc.declare_dram_parameter("wo", [HPC * DK, D], BF16, isOutput=False)
    d_ng = nc.declare_dram_parameter("negm", [128, KT_TILES], F32, isOutput=False)
    d_qs = nc.declare_dram_parameter("qmsc", [128, NQB * (QB // 128)], F32, isOutput=False)
    d_qv = nc.declare_dram_parameter("qbmv", [2, 128, NQ], BF16, isOutput=False)
    d_on = nc.declare_dram_parameter("ones", [1, DK], F32, isOutput=False)
    d_out = nc.declare_dram_parameter("out", [NQ, D], BF16, isOutput=True)

    QSC = QB // 128  # denominator scatter cols per q block

    with tile.TileContext(nc) as tc:
        with (
            tc.tile_pool(name="persist", bufs=1) as pp,
            tc.tile_pool(name="probs", bufs=3) as prp,
            tc.tile_pool(name="sb_small", bufs=2) as sp,
            tc.tile_pool(name="outsb", bufs=2) as op,
            tc.tile_pool(name="ps_sc", bufs=2, space="PSUM") as scp,
            tc.tile_pool(name="ps_at", bufs=1, space="PSUM") as atp,
        ):
            # ---- input DMAs: wq + x first (Q-projection is the critical path)
            t_wq = []
            for mc in range(MC):
                t = pp.tile([128, HPC * DK], BF16, name=f"wq{mc}", tag=f"wq{mc}")
                nc.sync.dma_start(out=t[:, :], in_=d_wq[mc * 128:(mc + 1) * 128, :])
                t_wq.append(t)
            t_xt = [pp.tile([128, NQ], BF16, name=f"xt{mc}", tag=f"xt{mc}")
                    for mc in range(MC)]
            for cb in range(NQ // 512):   # column-block-major so block 0 lands first
                for mc in range(MC):
                    nc.sync.dma_start(
                        out=t_xt[mc][:, cb * 512:(cb + 1) * 512],
                        in_=d_xt[mc * 128:(mc + 1) * 128, cb * 512:(cb + 1) * 512])

            # ---- remaining parameter DMAs (needed from phase B on)
            t_ng = pp.tile([128, KT_TILES], F32, name="negm", tag="negm")
            nc.sync.dma_start(out=t_ng[:, :], in_=d_ng[:, :])
            t_qs = pp.tile([128, NQB * QSC], F32, name="qmsc", tag="qmsc")
            nc.sync.dma_start(out=t_qs[:, :], in_=d_qs[:, :])
            t_on = pp.tile([1, DK], F32R, name="ones", tag="ones")
            nc.sync.dma_start(out=t_on[:, :], in_=d_on[:, :].bitcast(F32R))
            t_kt = []
            for p in range(2):
                t = pp.tile([128, NKV], BF16, name=f"kt{p}", tag=f"kt{p}")
                nc.sync.dma_start(out=t[:, :], in_=d_kt[p, :, :])
                t_kt.append(t)
            t_vx = []
            for h in range(HPC):
                t = pp.tile([128, KT_TILES * (DK + 1)], BF16, name=f"vx{h}", tag=f"vx{h}")
                nc.sync.dma_start(out=t[:, :], in_=d_vx[h, :, :])
                t_vx.append(t)
            t_wo = []
            for i in range(2):
                t = pp.tile([128, D], BF16, name=f"wo{i}", tag=f"wo{i}")
                nc.sync.dma_start(out=t[:, :], in_=d_wo[i * 128:(i + 1) * 128, :])
                t_wo.append(t)
            t_qv = []
            for p in range(2):
                t = pp.tile([128, NQ], BF16, name=f"qv{p}", tag=f"qv{p}")
                nc.sync.dma_start(out=t[:, :], in_=d_qv[p, :, :])
                t_qv.append(t)

            # persistent per-pair stacked Q^T and final attn^T tiles
            # (final tiles split per q block so phase C of block n never
            # false-shares with phase B writes of block n+1)
            t_qt = [pp.tile([128, NQ], BF16, name=f"qt{p}", tag=f"qt{p}") for p in range(2)]
            t_f = {(p, qh): pp.tile([128, QB], BF16, name=f"f{p}q{qh}", tag=f"f{p}q{qh}")
                   for p in range(2) for qh in range(NQB)}

            # ---- Phase A: Q^T = Wq_pair^T @ x^T, stacked per head pair ----
            for p in range(2):
                for half in range(NQ // QB):
                    ps_q = scp.tile([128, QB], F32, name="ps", tag="sc")
                    for nb in range(QB // 512):
                        cs = slice(half * QB + nb * 512, half * QB + (nb + 1) * 512)
                        for mc in range(MC):
                            nc.tensor.matmul(
                                ps_q[:, nb * 512:(nb + 1) * 512],
                                t_wq[mc][:, p * 128:(p + 1) * 128],
                                t_xt[mc][:, cs],
                                start=(mc == 0), stop=(mc == MC - 1),
                            )
                    nc.vector.tensor_copy(t_qt[p][:, half * QB:(half + 1) * QB], ps_q[:, :])

            # ---- Phase B + C interleaved per q block ----
            for qh in range(NQB):
                q0 = qh * QB
                for p in range(2):  # head pair (2p, 2p+1)
                    hA, hB = 2 * p, 2 * p + 1
                    at_ps = []
                    for hi in range(2):
                        at_ps.append(atp.tile([128, QB], F32, name=f"at{hi}",
                                              tag=f"at{hi}"))

                    def at_mms(kt, probs):
                        # attn matmuls one step behind scores so the PE FIFO
                        # never stalls waiting on exp(kt)
                        for hi in range(2):
                            for nb in range(QB // 512):
                                s = slice(nb * 512, (nb + 1) * 512)
                                nc.tensor.matmul(
                                    at_ps[hi][0:DK + 1, s],
                                    t_vx[2 * p + hi][:, kt * (DK + 1):(kt + 1) * (DK + 1)],
                                    probs[hi][:, s],
                                    start=(kt == 0), stop=(kt == KT_TILES - 1),
                                )

                    prev_probs = None
                    for kt in range(KT_TILES):
                        ks = slice(kt * 128, (kt + 1) * 128)
                        probs = []
                        for hi, hr in ((0, slice(0, DK)), (1, slice(DK, 128))):
                            sc = scp.tile([128, QB], F32, name="sc", tag="sc")
                            for nb in range(QB // 512):
                                s = slice(nb * 512, (nb + 1) * 512)
                                nc.tensor.matmul(
                                    sc[:, s], t_kt[p][hr, ks],
                                    t_qt[p][hr, q0 + nb * 512:q0 + (nb + 1) * 512],
                                    start=True, stop=True,
                                )
                            pt = prp.tile([128, QB], BF16, name=f"p{hi}", tag=f"p{hi}")
                            nc.scalar.activation(pt[:, :], sc[:, :], AF.Exp,
                                                 bias=t_ng[:, kt:kt + 1], scale=SCALE)
                            probs.append(pt)
                        if kt >= 1:
                            at_mms(kt - 1, prev_probs)
                        prev_probs = probs
                    at_mms(KT_TILES - 1, prev_probs)

                    # tail: denominators -> reciprocal -> broadcast -> finalize
                    for hi in range(2):
                        t_at = sp.tile([DK + 1, QB], F32, name="at_sb", tag="at_sb")
                        nc.vector.tensor_copy(t_at[:, :], at_ps[hi][0:DK + 1, :])
                        t_dsc = sp.tile([128, QSC], F32, name="dsc", tag="dsc")
                        nc.sync.dma_start(out=t_dsc[:, :], in_=t_at[DK:DK + 1, :])
                        t_drc = sp.tile([128, QSC], F32, name="drc", tag="drc")
                        nc.vector.reciprocal(t_drc[:, :], t_dsc[:, :])
                        t_dra = sp.tile([128, QSC], F32, name="dra", tag="dra")
                        nc.vector.tensor_mul(t_dra[:, :], t_drc[:, :],
                                             t_qs[:, qh * QSC:(qh + 1) * QSC])
                        t_ra = sp.tile([1, QB], F32R, name="ra", tag="ra")
                        nc.sync.dma_start(out=t_ra[:, :], in_=t_dra[:, :].bitcast(F32R))
                        ps_bc = scp.tile([128, QB], F32, name="bc", tag="sc")
                        for nb in range(QB // 512):
                            s = slice(nb * 512, (nb + 1) * 512)
                            nc.tensor.matmul(ps_bc[0:DK, s], t_on[:, :], t_ra[:, s],
                                             start=True, stop=True)
                        dst = t_f[(p, qh)][hi * DK:(hi + 1) * DK, :]
                        nc.vector.tensor_mul(dst, t_at[0:DK, :], ps_bc[0:DK, :])
                        nc.vector.tensor_add(dst, dst,
                                             t_qv[p][hi * DK:(hi + 1) * DK, q0:q0 + QB])

                # ---- Phase C for this q block ----
                for qt_i in range(QB // 128):
                    qs = slice(qt_i * 128, (qt_i + 1) * 128)
                    po = scp.tile([128, D], F32, name="po", tag="sc")
                    for nb in range(D // 512):
                        s = slice(nb * 512, (nb + 1) * 512)
                        nc.tensor.matmul(po[:, s], t_f[(0, qh)][:, qs], t_wo[0][:, s],
                                         start=True, stop=False)
                        nc.tensor.matmul(po[:, s], t_f[(1, qh)][:, qs], t_wo[1][:, s],
                                         start=False, stop=True)
                    t_out = op.tile([128, D], BF16, name="t_out", tag="t_out")
                    nc.vector.tensor_copy(t_out[:, :], po[:, :])
                    nc.sync.dma_start(out=d_out[q0 + qt_i * 128:q0 + (qt_i + 1) * 128, :],
                                      in_=t_out[:, :])

    _split_excess_waits(nc, limit=1)
    return nc


_PROGRAM = None


def _get_program():
    global _PROGRAM
    if _PROGRAM is None:
        _PROGRAM = _build_program()
    return _PROGRAM


def _core_inputs(c, x, K, V, Wq, Wo, kv_pad_mask, q_pad_mask):
    b = c // CPB
    g = c % CPB
    hs = slice(HPC * g, HPC * g + HPC)
    xt = np.ascontiguousarray(x[b].T).astype(BF)
    wq = np.ascontiguousarray(Wq[:, HPC * DK * g:HPC * DK * (g + 1)]).astype(BF)
    kth = K[b, hs].transpose(0, 2, 1)            # [4, DK, NKV]
    kt2 = np.ascontiguousarray(kth.reshape(2, 128, NKV)).astype(BF)
    v = V[b, hs].reshape(HPC, KT_TILES, 128, DK).transpose(0, 2, 1, 3)
    vext = np.concatenate(
        [v, np.ones((HPC, 128, KT_TILES, 1), np.float32)], axis=-1
    ).reshape(HPC, 128, KT_TILES * (DK + 1)).astype(BF)
    wo = np.ascontiguousarray(Wo[HPC * DK * g:HPC * DK * (g + 1), :]).astype(BF)
    kvm = kv_pad_mask[b, 0, 0].astype(bool)
    negm = np.ascontiguousarray(
        np.where(kvm, 0.0, -1e9).astype(np.float32).reshape(KT_TILES, 128).T)
    qm = q_pad_mask[b, 0, :, 0].astype(np.float32)            # [NQ]
    QSC = QB // 128
    qmsc = np.ascontiguousarray(
        qm.reshape(NQB, 128, QSC).transpose(1, 0, 2).reshape(128, NQB * QSC))
    mv = V[b, hs].mean(axis=1, dtype=np.float32)              # [4, DK]
    qbmv = (mv[:, :, None] * (1.0 - qm)[None, None, :]).reshape(2, 128, NQ).astype(BF)
    return dict(xt=xt, wq=wq, kt2=kt2, vext=vext, wo=wo, negm=negm,
                qmsc=qmsc, qbmv=qbmv, ones=np.ones((1, DK), np.float32))


def _install_ntff_hook():
    """The axon NTFF profile hook normally lives in antenv.axon_hooks,
    which this image lacks. Recreate it from trn_agent_boot so
    trace=True profiling works."""
    import sys
    import types
    try:
        from antenv.axon_hooks import get_axon_ntff_profile_hook  # noqa: F401
        return
    except ImportError:
        pass
    try:
        from trn_agent_boot.trn_boot import _ntff_profile_via_ctypes
        hook = _ntff_profile_via_ctypes("/opt/axon/libaxon_pjrt.so")
    except Exception:
        hook = None
    m = types.ModuleType("antenv.axon_hooks")
    m.get_axon_ntff_profile_hook = lambda: hook
    m.set_axon_ntff_profile_hook = lambda h: None
    sys.modules["antenv.axon_hooks"] = m


def kernel(x, K, V, Wq, Wo, kv_pad_mask, q_pad_mask, _trace=False):
    if _trace:
        _install_ntff_hook()
    nc = _get_program()
    x = np.asarray(x)
    K = np.asarray(K)
    V = np.asarray(V)
    Wq = np.asarray(Wq)
    Wo = np.asarray(Wo)
    kv_pad_mask = np.asarray(kv_pad_mask)
    q_pad_mask = np.asarray(q_pad_mask)
    in_maps = [_core_inputs(c, x, K, V, Wq, Wo, kv_pad_mask, q_pad_mask)
               for c in range(B * CPB)]
    res = run_bass_kernel_spmd(nc, in_maps, list(range(B * CPB)), trace=_trace)
    kernel._last_exec_ns = res.exec_time_ns
    kernel._last_results = res
    out = np.empty((B, NQ, D), np.float32)
    for b in range(B):
        acc = res.results[b * CPB]["out"].astype(np.float32)
        for j in range(1, CPB):
            acc = acc + res.results[b * CPB + j]["out"].astype(np.float32)
        out[b] = acc
    return out


kernel._last_exec_ns = None
kernel._last_results = None


# revision 16
# speedup vs baseline: 1.3187x; 1.3187x over previous
"""Cross multi-head attention on 8 Trainium2 NeuronCores.

Sharding: batch x head-group. Core c handles batch b = c//4 and heads
4*(c%4) .. 4*(c%4)+3. Wq tensor-sharded by head (columns), Wo by its
input (head) dim (rows); the 4 partial outputs per batch are summed on
the host.

v2 structure (all engine-level changes vs the 354us baseline):
  - all inputs bf16 (halves DMA), x streamed in 512-col blocks so the
    first Q-projection matmul starts after ~1MB of DMA instead of 9MB.
  - scores for a HEAD PAIR run concurrently in the PE array via row
    tiling: heads (2p, 2p+1) keep their K^T and Q^T stacked in
    partitions 0-63 / 64-127 of shared tiles; the two 64-contraction
    matmuls occupy disjoint row groups (tile_position (0,0) / (64,0)).
  - the per-block softmax denominators (row 64 of the attn PSUM, via
    the ones-column of V_ext) are DMA-scattered [1,1024]->[128,8],
    inverted with reciprocal_approx_fast (~100ns vs 6.5us serial DVE
    reciprocal that stalled the PE every block), multiplied by the
    scattered q-mask, and DMA-gathered back for the PE broadcast.
  - output projection + store run per q-block, interleaved with the
    next block's attention.
"""

import numpy as np
import ml_dtypes

import concourse.bass as bass
import concourse.mybir as mybir
import concourse.tile as tile
from concourse.bass_utils import run_bass_kernel_spmd

F32 = mybir.dt.float32
F32R = mybir.dt.float32r
BF16 = mybir.dt.bfloat16
AF = mybir.ActivationFunctionType
BF = ml_dtypes.bfloat16

B, H, NQ, NKV, D, DK = 2, 16, 2048, 2048, 1024, 64
HPC = 4              # heads per core
CPB = 4              # cores per batch
KT_TILES = NKV // 128
MC = D // 128        # model-dim chunks
QB = 1024            # q block width for scores/exp/attn
NQB = NQ // QB
SCALE = 1.0 / 8.0    # 1/sqrt(DK)


def _split_excess_waits(nc, limit=1):
    """This walrus build rejects instructions carrying several sem waits.
    Move excess waits onto standalone EventSemaphore instructions placed
    directly before the offender on the same (FIFO) engine queue."""
    n = 0
    for f in nc.m.functions:
        for bb in f.blocks:
            out = []
            for inst in bb.instructions:
                si = inst.sync_info
                waits = list(si.on_wait) if si is not None else []
                if len(waits) > limit:
                    excess, keep = waits[:-limit], waits[-limit:]
                    for w in excess:
                        n += 1
                        out.append(mybir.InstEventSemaphore(
                            name=f"wsplit-{n}-{inst.name}",
                            engine=inst.engine,
                            ins=[], outs=[],
                            sync_info=mybir.SyncInfo(on_wait=[w], on_update=[]),
                        ))
                    si.on_wait = keep
                out.append(inst)
            bb.instructions = out
    return n


def _build_program():
    nc = bass.Bass("TRN2", target_bir_lowering=False, debug=False, num_devices=8)

    d_wq = nc.declare_dram_parameter("wq", [D, HPC * DK], BF16, isOutput=False)
    d_xt = nc.declare_dram_parameter("xt", [D, NQ], BF16, isOutput=False)
    d_kt = nc.declare_dram_parameter("kt2", [2, 128, NKV], BF16, isOutput=False)
    d_vx = nc.declare_dram_parameter("vext", [HPC, 128, KT_TILES * (DK + 1)], BF16, isOutput=False)
    d_wo = nc.declare_dram_parameter("wo", [HPC * DK, D], BF16, isOutput=False)
    d_qs = nc.declare_dram_parameter("qmsc", [128, NQB * (QB // 128)], F32, isOutput=False)
    d_qv = nc.declare_dram_parameter("qbmv", [2, 128, NQ], BF16, isOutput=False)
    d_on = nc.declare_dram_parameter("ones", [1, DK], F32, isOutput=False)
    d_out = nc.declare_dram_parameter("out", [NQ, D], BF16, isOutput=True)

    QSC = QB // 128  # denominator scatter cols per q block

    with tile.TileContext(nc) as tc:
        with (
            tc.tile_pool(name="persist", bufs=1) as pp,
            tc.tile_pool(name="probs", bufs=3) as prp,
            tc.tile_pool(name="sb_small", bufs=2) as sp,
            tc.tile_pool(name="outsb", bufs=2) as op,
            tc.tile_pool(name="ps_sc", bufs=2, space="PSUM") as scp,
            tc.tile_pool(name="ps_at", bufs=1, space="PSUM") as atp,
        ):
            # ---- input DMAs: wq + x first (Q-projection is the critical path)
            t_wq = []
            for mc in range(MC):
                t = pp.tile([128, HPC * DK], BF16, name=f"wq{mc}", tag=f"wq{mc}")
                nc.sync.dma_start(out=t[:, :], in_=d_wq[mc * 128:(mc + 1) * 128, :])
                t_wq.append(t)
            t_xt = [pp.tile([128, NQ], BF16, name=f"xt{mc}", tag=f"xt{mc}")
                    for mc in range(MC)]
            for cb in range(NQ // 512):   # column-block-major so block 0 lands first
                for mc in range(MC):
                    nc.sync.dma_start(
                        out=t_xt[mc][:, cb * 512:(cb + 1) * 512],
                        in_=d_xt[mc * 128:(mc + 1) * 128, cb * 512:(cb + 1) * 512])

            # ---- remaining parameter DMAs (needed from phase B on)
            t_qs = pp.tile([128, NQB * QSC], F32, name="qmsc", tag="qmsc")
            nc.sync.dma_start(out=t_qs[:, :], in_=d_qs[:, :])
            t_on = pp.tile([1, DK], F32R, name="ones", tag="ones")
            nc.sync.dma_start(out=t_on[:, :], in_=d_on[:, :].bitcast(F32R))
            t_kt = []
            for p in range(2):
                t = pp.tile([128, NKV], BF16, name=f"kt{p}", tag=f"kt{p}")
                nc.sync.dma_start(out=t[:, :], in_=d_kt[p, :, :])
                t_kt.append(t)
            t_vx = []
            for h in range(HPC):
                t = pp.tile([128, KT_TILES * (DK + 1)], BF16, name=f"vx{h}", tag=f"vx{h}")
                nc.sync.dma_start(out=t[:, :], in_=d_vx[h, :, :])
                t_vx.append(t)
            t_wo = []
            for i in range(2):
                t = pp.tile([128, D], BF16, name=f"wo{i}", tag=f"wo{i}")
                nc.sync.dma_start(out=t[:, :], in_=d_wo[i * 128:(i + 1) * 128, :])
                t_wo.append(t)
            t_qv = []
            for p in range(2):
                t = pp.tile([128, NQ], BF16, name=f"qv{p}", tag=f"qv{p}")
                nc.sync.dma_start(out=t[:, :], in_=d_qv[p, :, :])
                t_qv.append(t)

            # persistent per-pair stacked Q^T and final attn^T tiles
            # (final tiles split per q block so phase C of block n never
            # false-shares with phase B writes of block n+1)
            t_qt = [pp.tile([128, NQ], BF16, name=f"qt{p}", tag=f"qt{p}") for p in range(2)]
            t_f = {(p, qh): pp.tile([128, QB], BF16, name=f"f{p}q{qh}", tag=f"f{p}q{qh}")
                   for p in range(2) for qh in range(NQB)}

            # ---- Phase A: Q^T = Wq_pair^T @ x^T, stacked per head pair ----
            for p in range(2):
                for half in range(NQ // QB):
                    ps_q = scp.tile([128, QB], F32, name="ps", tag="sc")
                    for nb in range(QB // 512):
                        cs = slice(half * QB + nb * 512, half * QB + (nb + 1) * 512)
                        for mc in range(MC):
                            nc.tensor.matmul(
                                ps_q[:, nb * 512:(nb + 1) * 512],
                                t_wq[mc][:, p * 128:(p + 1) * 128],
                                t_xt[mc][:, cs],
                                start=(mc == 0), stop=(mc == MC - 1),
                            )
                    nc.vector.tensor_copy(t_qt[p][:, half * QB:(half + 1) * QB], ps_q[:, :])

            # ---- Phase B + C, software-pipelined across q blocks ----
            def attn_block(qh, p):
                """Scores + exp + attn + normalization tail for head pair
                (2p, 2p+1) on q block qh."""
                q0 = qh * QB
                at_ps = [atp.tile([128, QB], F32, name=f"at{hi}", tag=f"at{hi}")
                         for hi in range(2)]

                def at_mms(kt, probs):
                    # attn matmuls one step behind scores so the PE FIFO
                    # never stalls waiting on exp(kt)
                    for hi in range(2):
                        for nb in range(QB // 512):
                            s = slice(nb * 512, (nb + 1) * 512)
                            nc.tensor.matmul(
                                at_ps[hi][0:DK + 1, s],
                                t_vx[2 * p + hi][:, kt * (DK + 1):(kt + 1) * (DK + 1)],
                                probs[hi][:, s],
                                start=(kt == 0), stop=(kt == KT_TILES - 1),
                            )

                prev_probs = None
                for kt in range(KT_TILES):
                    ks = slice(kt * 128, (kt + 1) * 128)
                    sc = []
                    for hi, hr in ((0, slice(0, DK)), (1, slice(DK, 128))):
                        sc.append(scp.tile([128, QB], F32, name="sc", tag="sc"))
                    # emit the head pair's matmuls adjacently per 512-col
                    # chunk: disjoint PE row groups -> concurrent execution
                    for nb in range(QB // 512):
                        s = slice(nb * 512, (nb + 1) * 512)
                        for hi, hr in ((0, slice(0, DK)), (1, slice(DK, 128))):
                            nc.tensor.matmul(
                                sc[hi][:, s], t_kt[p][hr, ks],
                                t_qt[p][hr, q0 + nb * 512:q0 + (nb + 1) * 512],
                                start=True, stop=True,
                            )
                    probs = []
                    for hi in range(2):
                        pt = prp.tile([128, QB], BF16, name=f"p{hi}", tag=f"p{hi}")
                        nc.scalar.activation(pt[:, :], sc[hi][:, :], AF.Exp)
                        probs.append(pt)
                    if kt >= 1:
                        at_mms(kt - 1, prev_probs)
                    prev_probs = probs
                at_mms(KT_TILES - 1, prev_probs)

                # tail: denominators -> reciprocal -> broadcast -> finalize.
                # The broadcast matmul reuses this block's own attn PSUM tile
                # (free after the SBUF copy) so it never blocks the scores
                # ring of the next block.
                for hi in range(2):
                    t_at = sp.tile([DK + 1, QB], F32, name="at_sb", tag="at_sb")
                    nc.vector.tensor_copy(t_at[:, :], at_ps[hi][0:DK + 1, :])
                    t_dsc = sp.tile([128, QSC], F32, name="dsc", tag="dsc")
                    nc.sync.dma_start(out=t_dsc[:, :], in_=t_at[DK:DK + 1, :])
                    t_drc = sp.tile([128, QSC], F32, name="drc", tag="drc")
                    nc.vector.reciprocal(t_drc[:, :], t_dsc[:, :])
                    t_dra = sp.tile([128, QSC], F32, name="dra", tag="dra")
                    nc.vector.tensor_mul(t_dra[:, :], t_drc[:, :],
                                         t_qs[:, qh * QSC:(qh + 1) * QSC])
                    t_ra = sp.tile([1, QB], F32R, name="ra", tag="ra")
                    nc.sync.dma_start(out=t_ra[:, :], in_=t_dra[:, :].bitcast(F32R))
                    for nb in range(QB // 512):
                        s = slice(nb * 512, (nb + 1) * 512)
                        nc.tensor.matmul(at_ps[hi][0:DK, s], t_on[:, :], t_ra[:, s],
                                         start=True, stop=True)
                    dst = t_f[(p, qh)][hi * DK:(hi + 1) * DK, :]
                    nc.vector.tensor_mul(dst, t_at[0:DK, :], at_ps[hi][0:DK, :])
                    nc.vector.tensor_add(dst, dst,
                                         t_qv[p][hi * DK:(hi + 1) * DK, q0:q0 + QB])

            def phase_c(qh):
                q0 = qh * QB
                for qt_i in range(QB // 128):
                    qs = slice(qt_i * 128, (qt_i + 1) * 128)
                    po = scp.tile([128, D], F32, name="po", tag="sc")
                    for nb in range(D // 512):
                        s = slice(nb * 512, (nb + 1) * 512)
                        nc.tensor.matmul(po[:, s], t_f[(0, qh)][:, qs], t_wo[0][:, s],
                                         start=True, stop=False)
                        nc.tensor.matmul(po[:, s], t_f[(1, qh)][:, qs], t_wo[1][:, s],
                                         start=False, stop=True)
                    t_out = op.tile([128, D], BF16, name="t_out", tag="t_out")
                    nc.vector.tensor_copy(t_out[:, :], po[:, :])
                    nc.sync.dma_start(out=d_out[q0 + qt_i * 128:q0 + (qt_i + 1) * 128, :],
                                      in_=t_out[:, :])

            # phase C of block qh is emitted after the first half of block
            # qh+1, so its matmuls never wait on the (qh, p=1) tail chain
            attn_block(0, 0)
            attn_block(0, 1)
            attn_block(1, 0)
            phase_c(0)
            attn_block(1, 1)
            phase_c(1)

    _split_excess_waits(nc, limit=1)
    return nc


_PROGRAM = None


def _get_program():
    global _PROGRAM
    if _PROGRAM is None:
        _PROGRAM = _build_program()
    return _PROGRAM


def _core_inputs(c, x, K, V, Wq, Wo, kv_pad_mask, q_pad_mask):
    b = c // CPB
    g = c % CPB
    hs = slice(HPC * g, HPC * g + HPC)
    xt = np.ascontiguousarray(x[b].T).astype(BF)
    wq = np.ascontiguousarray(Wq[:, HPC * DK * g:HPC * DK * (g + 1)]).astype(BF)
    kth = K[b, hs].transpose(0, 2, 1) * np.float32(SCALE)     # [4, DK, NKV]
    kt2 = np.ascontiguousarray(kth.reshape(2, 128, NKV)).astype(BF)
    kvm = kv_pad_mask[b, 0, 0].astype(bool)
    # kv mask folded into V_ext: zeroed rows (incl. the ones column) drop
    # masked kv from both the attn accumulation and the denominator, so
    # exp needs no mask bias at all
    v = V[b, hs] * kvm[None, :, None]
    v = v.reshape(HPC, KT_TILES, 128, DK).transpose(0, 2, 1, 3)
    ones = np.broadcast_to(
        kvm.reshape(KT_TILES, 128).T.astype(np.float32)[None, :, :, None],
        (HPC, 128, KT_TILES, 1))
    vext = np.concatenate([v, ones], axis=-1).reshape(
        HPC, 128, KT_TILES * (DK + 1)).astype(BF)
    wo = np.ascontiguousarray(Wo[HPC * DK * g:HPC * DK * (g + 1), :]).astype(BF)
    qm = q_pad_mask[b, 0, :, 0].astype(np.float32)            # [NQ]
    QSC = QB // 128
    qmsc = np.ascontiguousarray(
        qm.reshape(NQB, 128, QSC).transpose(1, 0, 2).reshape(128, NQB * QSC))
    mv = V[b, hs].mean(axis=1, dtype=np.float32)              # [4, DK]
    qbmv = (mv[:, :, None] * (1.0 - qm)[None, None, :]).reshape(2, 128, NQ).astype(BF)
    return dict(xt=xt, wq=wq, kt2=kt2, vext=vext, wo=wo,
                qmsc=qmsc, qbmv=qbmv, ones=np.ones((1, DK), np.float32))


def _install_ntff_hook():
    """The axon NTFF profile hook normally lives in antenv.axon_hooks,
    which this image lacks. Recreate it from trn_agent_boot so
    trace=True profiling works."""
    import sys
    import types
    try:
        from antenv.axon_hooks import get_axon_ntff_profile_hook  # noqa: F401
        return
    except ImportError:
        pass
    try:
        from trn_agent_boot.trn_boot import _ntff_profile_via_ctypes
        hook = _ntff_profile_via_ctypes("/opt/axon/libaxon_pjrt.so")
    except Exception:
        hook = None
    m = types.ModuleType("antenv.axon_hooks")
    m.get_axon_ntff_profile_hook = lambda: hook
    m.set_axon_ntff_profile_hook = lambda h: None
    sys.modules["antenv.axon_hooks"] = m


def kernel(x, K, V, Wq, Wo, kv_pad_mask, q_pad_mask, _trace=False):
    if _trace:
        _install_ntff_hook()
    nc = _get_program()
    x = np.asarray(x)
    K = np.asarray(K)
    V = np.asarray(V)
    Wq = np.asarray(Wq)
    Wo = np.asarray(Wo)
    kv_pad_mask = np.asarray(kv_pad_mask)
    q_pad_mask = np.asarray(q_pad_mask)
    in_maps = [_core_inputs(c, x, K, V, Wq, Wo, kv_pad_mask, q_pad_mask)
               for c in range(B * CPB)]
    res = run_bass_kernel_spmd(nc, in_maps, list(range(B * CPB)), trace=_trace)
    kernel._last_exec_ns = res.exec_time_ns
    kernel._last_results = res
    out = np.empty((B, NQ, D), np.float32)
    for b in range(B):
        acc = res.results[b * CPB]["out"].astype(np.float32)
        for j in range(1, CPB):
            acc = acc + res.results[b * CPB + j]["out"].astype(np.float32)
        out[b] = acc
    return out


kernel._last_exec_ns = None
kernel._last_results = None


# revision 25
# speedup vs baseline: 1.3353x; 1.0126x over previous
"""Cross multi-head attention on 8 Trainium2 NeuronCores.

Sharding: batch x head-group. Core c handles batch b = c//4 and heads
4*(c%4) .. 4*(c%4)+3. Wq tensor-sharded by head (columns), Wo by its
input (head) dim (rows); the 4 partial outputs per batch are summed on
the host.

v2 structure (all engine-level changes vs the 354us baseline):
  - all inputs bf16 (halves DMA), x streamed in 512-col blocks so the
    first Q-projection matmul starts after ~1MB of DMA instead of 9MB.
  - scores for a HEAD PAIR run concurrently in the PE array via row
    tiling: heads (2p, 2p+1) keep their K^T and Q^T stacked in
    partitions 0-63 / 64-127 of shared tiles; the two 64-contraction
    matmuls occupy disjoint row groups (tile_position (0,0) / (64,0)).
  - the per-block softmax denominators (row 64 of the attn PSUM, via
    the ones-column of V_ext) are DMA-scattered [1,1024]->[128,8],
    inverted with reciprocal_approx_fast (~100ns vs 6.5us serial DVE
    reciprocal that stalled the PE every block), multiplied by the
    scattered q-mask, and DMA-gathered back for the PE broadcast.
  - output projection + store run per q-block, interleaved with the
    next block's attention.
"""

import numpy as np
import ml_dtypes

import concourse.bass as bass
import concourse.mybir as mybir
import concourse.tile as tile
from concourse.bass_utils import run_bass_kernel_spmd

F32 = mybir.dt.float32
F32R = mybir.dt.float32r
BF16 = mybir.dt.bfloat16
AF = mybir.ActivationFunctionType
BF = ml_dtypes.bfloat16

B, H, NQ, NKV, D, DK = 2, 16, 2048, 2048, 1024, 64
HPC = 4              # heads per core
CPB = 4              # cores per batch
KT_TILES = NKV // 128
MC = D // 128        # model-dim chunks
QB = 1024            # q block width for scores/exp/attn
NQB = NQ // QB
SCALE = 1.0 / 8.0    # 1/sqrt(DK)


def _split_excess_waits(nc, limit=1):
    """This walrus build rejects instructions carrying several sem waits.
    Move excess waits onto standalone EventSemaphore instructions placed
    directly before the offender on the same (FIFO) engine queue."""
    n = 0
    for f in nc.m.functions:
        for bb in f.blocks:
            out = []
            for inst in bb.instructions:
                si = inst.sync_info
                waits = list(si.on_wait) if si is not None else []
                if len(waits) > limit:
                    excess, keep = waits[:-limit], waits[-limit:]
                    for w in excess:
                        n += 1
                        out.append(mybir.InstEventSemaphore(
                            name=f"wsplit-{n}-{inst.name}",
                            engine=inst.engine,
                            ins=[], outs=[],
                            sync_info=mybir.SyncInfo(on_wait=[w], on_update=[]),
                        ))
                    si.on_wait = keep
                out.append(inst)
            bb.instructions = out
    return n


def _build_program():
    nc = bass.Bass("TRN2", target_bir_lowering=False, debug=False, num_devices=8)

    d_wq = nc.declare_dram_parameter("wq", [D, HPC * DK], BF16, isOutput=False)
    d_xt = nc.declare_dram_parameter("xt", [D, NQ], BF16, isOutput=False)
    d_kt = nc.declare_dram_parameter("kt2", [2, 128, NKV], BF16, isOutput=False)
    d_vx = nc.declare_dram_parameter("vext", [HPC, 128, KT_TILES * (DK + 1)], BF16, isOutput=False)
    d_wo = nc.declare_dram_parameter("wo", [HPC * DK, D], BF16, isOutput=False)
    d_qs = nc.declare_dram_parameter("qmsc", [128, NQB * (QB // 128)], F32, isOutput=False)
    d_qv = nc.declare_dram_parameter("qbmv", [2, 128, NQ], BF16, isOutput=False)
    d_on = nc.declare_dram_parameter("ones", [1, DK], F32, isOutput=False)
    d_out = nc.declare_dram_parameter("out", [NQ, D], BF16, isOutput=True)

    QSC = QB // 128  # denominator scatter cols per q block

    with tile.TileContext(nc) as tc:
        with (
            tc.tile_pool(name="persist", bufs=1) as pp,
            tc.tile_pool(name="probs", bufs=3) as prp,
            tc.tile_pool(name="sb_small", bufs=2) as sp,
            tc.tile_pool(name="outsb", bufs=2) as op,
            tc.tile_pool(name="ps_sc", bufs=2, space="PSUM") as scp,
            tc.tile_pool(name="ps_at", bufs=1, space="PSUM") as atp,
        ):
            # ---- input DMAs: wq + x first (Q-projection is the critical path)
            t_wq = []
            for mc in range(MC):
                t = pp.tile([128, HPC * DK], BF16, name=f"wq{mc}", tag=f"wq{mc}")
                nc.sync.dma_start(out=t[:, :], in_=d_wq[mc * 128:(mc + 1) * 128, :])
                t_wq.append(t)
            t_xt = [pp.tile([128, NQ], BF16, name=f"xt{mc}", tag=f"xt{mc}")
                    for mc in range(MC)]
            for cb in range(NQ // 512):   # column-block-major so block 0 lands first
                for mc in range(MC):
                    nc.sync.dma_start(
                        out=t_xt[mc][:, cb * 512:(cb + 1) * 512],
                        in_=d_xt[mc * 128:(mc + 1) * 128, cb * 512:(cb + 1) * 512])

            # ---- remaining parameter DMAs (needed from phase B on)
            t_on = pp.tile([1, DK], F32R, name="ones", tag="ones")
            nc.sync.dma_start(out=t_on[:, :], in_=d_on[:, :].bitcast(F32R))
            t_qs = pp.tile([128, NQB * QSC], F32, name="qmsc", tag="qmsc")
            nc.sync.dma_start(out=t_qs[:, :], in_=d_qs[:, :])
            t_kt = []
            for p in range(2):
                t = pp.tile([128, NKV], BF16, name=f"kt{p}", tag=f"kt{p}")
                nc.sync.dma_start(out=t[:, :], in_=d_kt[p, :, :])
                t_kt.append(t)
            t_vx = []
            for h in range(HPC):
                t = pp.tile([128, KT_TILES * (DK + 1)], BF16, name=f"vx{h}", tag=f"vx{h}")
                nc.sync.dma_start(out=t[:, :], in_=d_vx[h, :, :])
                t_vx.append(t)
            t_wo = []
            for i in range(2):
                t = pp.tile([128, D], BF16, name=f"wo{i}", tag=f"wo{i}")
                nc.sync.dma_start(out=t[:, :], in_=d_wo[i * 128:(i + 1) * 128, :])
                t_wo.append(t)
            t_qv = []
            for p in range(2):
                t = pp.tile([128, NQ], BF16, name=f"qv{p}", tag=f"qv{p}")
                nc.sync.dma_start(out=t[:, :], in_=d_qv[p, :, :])
                t_qv.append(t)

            # persistent per-pair stacked Q^T and final attn^T tiles
            # (final tiles split per q block so phase C of block n never
            # false-shares with phase B writes of block n+1)
            t_qt = [pp.tile([128, NQ], BF16, name=f"qt{p}", tag=f"qt{p}") for p in range(2)]
            t_f = {(p, qh): pp.tile([128, QB], BF16, name=f"f{p}q{qh}", tag=f"f{p}q{qh}")
                   for p in range(2) for qh in range(NQB)}

            # ---- Phase A: Q^T = Wq_pair^T @ x^T, stacked per head pair ----
            for p in range(2):
                for half in range(NQ // QB):
                    ps_q = scp.tile([128, QB], F32, name="ps", tag="sc")
                    for nb in range(QB // 512):
                        cs = slice(half * QB + nb * 512, half * QB + (nb + 1) * 512)
                        for mc in range(MC):
                            nc.tensor.matmul(
                                ps_q[:, nb * 512:(nb + 1) * 512],
                                t_wq[mc][:, p * 128:(p + 1) * 128],
                                t_xt[mc][:, cs],
                                start=(mc == 0), stop=(mc == MC - 1),
                            )
                    nc.vector.tensor_copy(t_qt[p][:, half * QB:(half + 1) * QB], ps_q[:, :])

            # ---- Phase B + C, software-pipelined across q blocks ----
            def attn_block(qh, p):
                """Scores + exp + attn + normalization tail for head pair
                (2p, 2p+1) on q block qh."""
                q0 = qh * QB
                at_ps = [atp.tile([128, QB], F32, name=f"at{hi}", tag=f"at{hi}")
                         for hi in range(2)]

                def at_mms(kt, probs):
                    # attn matmuls one step behind scores so the PE FIFO
                    # never stalls waiting on exp(kt)
                    for hi in range(2):
                        for nb in range(QB // 512):
                            s = slice(nb * 512, (nb + 1) * 512)
                            nc.tensor.matmul(
                                at_ps[hi][0:DK + 1, s],
                                t_vx[2 * p + hi][:, kt * (DK + 1):(kt + 1) * (DK + 1)],
                                probs[hi][:, s],
                                start=(kt == 0), stop=(kt == KT_TILES - 1),
                            )

                prev_probs = None
                for kt in range(KT_TILES):
                    ks = slice(kt * 128, (kt + 1) * 128)
                    sc = []
                    probs = []
                    for hi in range(2):
                        sc.append(scp.tile([128, QB], F32, name="sc", tag="sc"))
                    for nb in range(QB // 512):
                        s = slice(nb * 512, (nb + 1) * 512)
                        for hi, hr in ((0, slice(0, DK)), (1, slice(DK, 128))):
                            nc.tensor.matmul(
                                sc[hi][:, s], t_kt[p][hr, ks],
                                t_qt[p][hr, q0 + nb * 512:q0 + (nb + 1) * 512],
                                start=True, stop=True,
                            )
                    for hi in range(2):
                        pt = prp.tile([128, QB], BF16, name=f"p{hi}", tag=f"p{hi}")
                        nc.scalar.activation(pt[:, :], sc[hi][:, :], AF.Exp)
                        probs.append(pt)
                    if kt >= 1:
                        at_mms(kt - 1, prev_probs)
                    prev_probs = probs
                at_mms(KT_TILES - 1, prev_probs)

                # tail: denominators -> reciprocal -> broadcast (GpSimd, fully
                # off the PE) -> finalize
                for hi in range(2):
                    t_at = sp.tile([DK + 1, QB], F32, name="at_sb", tag="at_sb")
                    nc.vector.tensor_copy(t_at[:, :], at_ps[hi][0:DK + 1, :])
                    t_dsc = sp.tile([128, QSC], F32, name="dsc", tag="dsc")
                    nc.sync.dma_start(out=t_dsc[:, :], in_=t_at[DK:DK + 1, :])
                    t_drc = sp.tile([128, QSC], F32, name="drc", tag="drc")
                    nc.vector.reciprocal(t_drc[:, :], t_dsc[:, :])
                    t_dra = sp.tile([128, QSC], F32, name="dra", tag="dra")
                    nc.vector.tensor_mul(t_dra[:, :], t_drc[:, :],
                                         t_qs[:, qh * QSC:(qh + 1) * QSC])
                    t_ra = sp.tile([1, QB], F32R, name="ra", tag="ra")
                    nc.sync.dma_start(out=t_ra[:, :], in_=t_dra[:, :].bitcast(F32R))
                    for nb in range(QB // 512):
                        s = slice(nb * 512, (nb + 1) * 512)
                        nc.tensor.matmul(at_ps[hi][0:DK, s], t_on[:, :], t_ra[:, s],
                                         start=True, stop=True)
                    dst = t_f[(p, qh)][hi * DK:(hi + 1) * DK, :]
                    nc.vector.tensor_mul(dst, t_at[0:DK, :], at_ps[hi][0:DK, :])
                    nc.vector.tensor_add(dst, dst,
                                         t_qv[p][hi * DK:(hi + 1) * DK, q0:q0 + QB])

            def phase_c(qh):
                q0 = qh * QB
                for qt_i in range(QB // 128):
                    qs = slice(qt_i * 128, (qt_i + 1) * 128)
                    po = scp.tile([128, D], F32, name="po", tag="sc")
                    for nb in range(D // 512):
                        s = slice(nb * 512, (nb + 1) * 512)
                        nc.tensor.matmul(po[:, s], t_f[(0, qh)][:, qs], t_wo[0][:, s],
                                         start=True, stop=False)
                        nc.tensor.matmul(po[:, s], t_f[(1, qh)][:, qs], t_wo[1][:, s],
                                         start=False, stop=True)
                    t_out = op.tile([128, D], BF16, name="t_out", tag="t_out")
                    nc.vector.tensor_copy(t_out[:, :], po[:, :])
                    nc.sync.dma_start(out=d_out[q0 + qt_i * 128:q0 + (qt_i + 1) * 128, :],
                                      in_=t_out[:, :])

            # phase C of block qh is emitted after the first half of block
            # qh+1, so its matmuls never wait on the (qh, p=1) tail chain
            attn_block(0, 0)
            attn_block(0, 1)
            attn_block(1, 0)
            phase_c(0)
            attn_block(1, 1)
            phase_c(1)

    _split_excess_waits(nc, limit=1)
    return nc


_PROGRAM = None


def _get_program():
    global _PROGRAM
    if _PROGRAM is None:
        _PROGRAM = _build_program()
    return _PROGRAM


def _core_inputs(c, x, K, V, Wq, Wo, kv_pad_mask, q_pad_mask):
    b = c // CPB
    g = c % CPB
    hs = slice(HPC * g, HPC * g + HPC)
    xt = np.ascontiguousarray(x[b].T).astype(BF)
    wq = np.ascontiguousarray(Wq[:, HPC * DK * g:HPC * DK * (g + 1)]).astype(BF)
    kth = K[b, hs].transpose(0, 2, 1) * np.float32(SCALE)     # [4, DK, NKV]
    kt2 = np.ascontiguousarray(kth.reshape(2, 128, NKV)).astype(BF)
    kvm = kv_pad_mask[b, 0, 0].astype(bool)
    # kv mask folded into V_ext: zeroed rows (incl. the ones column) drop
    # masked kv from both the attn accumulation and the denominator, so
    # exp needs no mask bias at all
    v = V[b, hs] * kvm[None, :, None]
    v = v.reshape(HPC, KT_TILES, 128, DK).transpose(0, 2, 1, 3)
    ones = np.broadcast_to(
        kvm.reshape(KT_TILES, 128).T.astype(np.float32)[None, :, :, None],
        (HPC, 128, KT_TILES, 1))
    vext = np.concatenate([v, ones], axis=-1).reshape(
        HPC, 128, KT_TILES * (DK + 1)).astype(BF)
    wo = np.ascontiguousarray(Wo[HPC * DK * g:HPC * DK * (g + 1), :]).astype(BF)
    qm = q_pad_mask[b, 0, :, 0].astype(np.float32)            # [NQ]
    QSC = QB // 128
    qmsc = np.ascontiguousarray(
        qm.reshape(NQB, 128, QSC).transpose(1, 0, 2).reshape(128, NQB * QSC))
    mv = V[b, hs].mean(axis=1, dtype=np.float32)              # [4, DK]
    qbmv = (mv[:, :, None] * (1.0 - qm)[None, None, :]).reshape(2, 128, NQ).astype(BF)
    return dict(xt=xt, wq=wq, kt2=kt2, vext=vext, wo=wo,
                qmsc=qmsc, qbmv=qbmv, ones=np.ones((1, DK), np.float32))


def _install_ntff_hook():
    """The axon NTFF profile hook normally lives in antenv.axon_hooks,
    which this image lacks. Recreate it from trn_agent_boot so
    trace=True profiling works."""
    import sys
    import types
    try:
        from antenv.axon_hooks import get_axon_ntff_profile_hook  # noqa: F401
        return
    except ImportError:
        pass
    try:
        from trn_agent_boot.trn_boot import _ntff_profile_via_ctypes
        hook = _ntff_profile_via_ctypes("/opt/axon/libaxon_pjrt.so")
    except Exception:
        hook = None
    m = types.ModuleType("antenv.axon_hooks")
    m.get_axon_ntff_profile_hook = lambda: hook
    m.set_axon_ntff_profile_hook = lambda h: None
    sys.modules["antenv.axon_hooks"] = m


def kernel(x, K, V, Wq, Wo, kv_pad_mask, q_pad_mask, _trace=False):
    if _trace:
        _install_ntff_hook()
    nc = _get_program()
    x = np.asarray(x)
    K = np.asarray(K)
    V = np.asarray(V)
    Wq = np.asarray(Wq)
    Wo = np.asarray(Wo)
    kv_pad_mask = np.asarray(kv_pad_mask)
    q_pad_mask = np.asarray(q_pad_mask)
    in_maps = [_core_inputs(c, x, K, V, Wq, Wo, kv_pad_mask, q_pad_mask)
               for c in range(B * CPB)]
    res = run_bass_kernel_spmd(nc, in_maps, list(range(B * CPB)), trace=_trace)
    kernel._last_exec_ns = res.exec_time_ns
    kernel._last_results = res
    out = np.empty((B, NQ, D), np.float32)
    for b in range(B):
        acc = res.results[b * CPB]["out"].astype(np.float32)
        for j in range(1, CPB):
            acc = acc + res.results[b * CPB + j]["out"].astype(np.float32)
        out[b] = acc
    return out


kernel._last_exec_ns = None
kernel._last_results = None


# revision 28
# speedup vs baseline: 1.4550x; 1.0897x over previous
"""Cross multi-head attention on 8 Trainium2 NeuronCores.

Sharding: batch x head-group. Core c handles batch b = c//4 and heads
4*(c%4) .. 4*(c%4)+3. Wq tensor-sharded by head (columns), Wo by its
input (head) dim (rows); the 4 partial outputs per batch are summed on
the host.

v2 structure (all engine-level changes vs the 354us baseline):
  - all inputs bf16 (halves DMA), x streamed in 512-col blocks so the
    first Q-projection matmul starts after ~1MB of DMA instead of 9MB.
  - scores for a HEAD PAIR run concurrently in the PE array via row
    tiling: heads (2p, 2p+1) keep their K^T and Q^T stacked in
    partitions 0-63 / 64-127 of shared tiles; the two 64-contraction
    matmuls occupy disjoint row groups (tile_position (0,0) / (64,0)).
  - the per-block softmax denominators (row 64 of the attn PSUM, via
    the ones-column of V_ext) are DMA-scattered [1,1024]->[128,8],
    inverted with reciprocal_approx_fast (~100ns vs 6.5us serial DVE
    reciprocal that stalled the PE every block), multiplied by the
    scattered q-mask, and DMA-gathered back for the PE broadcast.
  - output projection + store run per q-block, interleaved with the
    next block's attention.
"""

import numpy as np
import ml_dtypes

import concourse.bass as bass
import concourse.mybir as mybir
import concourse.tile as tile
from concourse.bass_utils import run_bass_kernel_spmd

F32 = mybir.dt.float32
F32R = mybir.dt.float32r
BF16 = mybir.dt.bfloat16
AF = mybir.ActivationFunctionType
BF = ml_dtypes.bfloat16

B, H, NQ, NKV, D, DK = 2, 16, 2048, 2048, 1024, 64
HPC = 4              # heads per core
CPB = 4              # cores per batch
KT_TILES = NKV // 128
MC = D // 128        # model-dim chunks
QB = 1024            # q block width for scores/exp/attn
NQB = NQ // QB
SCALE = 1.0 / 8.0    # 1/sqrt(DK)


def _split_excess_waits(nc, limit=1):
    """This walrus build rejects instructions carrying several sem waits.
    Move excess waits onto standalone EventSemaphore instructions placed
    directly before the offender on the same (FIFO) engine queue."""
    n = 0
    for f in nc.m.functions:
        for bb in f.blocks:
            out = []
            for inst in bb.instructions:
                si = inst.sync_info
                waits = list(si.on_wait) if si is not None else []
                if len(waits) > limit:
                    excess, keep = waits[:-limit], waits[-limit:]
                    for w in excess:
                        n += 1
                        out.append(mybir.InstEventSemaphore(
                            name=f"wsplit-{n}-{inst.name}",
                            engine=inst.engine,
                            ins=[], outs=[],
                            sync_info=mybir.SyncInfo(on_wait=[w], on_update=[]),
                        ))
                    si.on_wait = keep
                out.append(inst)
            bb.instructions = out
    return n


def _build_program():
    nc = bass.Bass("TRN2", target_bir_lowering=False, debug=False, num_devices=8)

    d_wq = nc.declare_dram_parameter("wq", [D, HPC * DK], BF16, isOutput=False)
    d_xt = nc.declare_dram_parameter("xt", [D, NQ], BF16, isOutput=False)
    d_kt = nc.declare_dram_parameter("kt2", [2, 128, NKV], BF16, isOutput=False)
    d_vx = nc.declare_dram_parameter("vext", [HPC, 128, KT_TILES * (DK + 1)], BF16, isOutput=False)
    d_wo = nc.declare_dram_parameter("wo", [HPC * DK, D], BF16, isOutput=False)
    d_qs = nc.declare_dram_parameter("qmsc", [128, NQB * (QB // 128)], F32, isOutput=False)
    d_qv = nc.declare_dram_parameter("qbmv", [2, 128, NQ], BF16, isOutput=False)
    d_on = nc.declare_dram_parameter("ones", [1, DK], F32, isOutput=False)
    d_out = nc.declare_dram_parameter("out", [NQ, D], BF16, isOutput=True)

    QSC = QB // 128  # denominator scatter cols per q block

    with tile.TileContext(nc) as tc:
        with (
            tc.tile_pool(name="persist", bufs=1) as pp,
            tc.tile_pool(name="probs", bufs=4) as prp,
            tc.tile_pool(name="sb_small", bufs=2) as sp,
            tc.tile_pool(name="outsb", bufs=2) as op,
            tc.tile_pool(name="ps_sc", bufs=2, space="PSUM") as scp,
            tc.tile_pool(name="ps_at", bufs=1, space="PSUM") as atp,
        ):
            # ---- input DMAs: wq + x first (Q-projection is the critical path)
            t_wq = []
            for mc in range(MC):
                t = pp.tile([128, HPC * DK], BF16, name=f"wq{mc}", tag=f"wq{mc}")
                nc.sync.dma_start(out=t[:, :], in_=d_wq[mc * 128:(mc + 1) * 128, :])
                t_wq.append(t)
            t_xt = [pp.tile([128, NQ], BF16, name=f"xt{mc}", tag=f"xt{mc}")
                    for mc in range(MC)]
            for cb in range(NQ // 512):   # column-block-major so block 0 lands first
                for mc in range(MC):
                    nc.sync.dma_start(
                        out=t_xt[mc][:, cb * 512:(cb + 1) * 512],
                        in_=d_xt[mc * 128:(mc + 1) * 128, cb * 512:(cb + 1) * 512])

            # ---- remaining parameter DMAs (needed from phase B on)
            t_on = pp.tile([1, DK], F32R, name="ones", tag="ones")
            nc.sync.dma_start(out=t_on[:, :], in_=d_on[:, :].bitcast(F32R))
            t_qs = pp.tile([128, NQB * QSC], F32, name="qmsc", tag="qmsc")
            nc.sync.dma_start(out=t_qs[:, :], in_=d_qs[:, :])
            t_kt = []
            for p in range(2):
                t = pp.tile([128, NKV], BF16, name=f"kt{p}", tag=f"kt{p}")
                nc.sync.dma_start(out=t[:, :], in_=d_kt[p, :, :])
                t_kt.append(t)
            t_vx = []
            for h in range(HPC):
                t = pp.tile([128, KT_TILES * (DK + 1)], BF16, name=f"vx{h}", tag=f"vx{h}")
                nc.sync.dma_start(out=t[:, :], in_=d_vx[h, :, :])
                t_vx.append(t)
            t_wo = []
            for i in range(2):
                t = pp.tile([128, D], BF16, name=f"wo{i}", tag=f"wo{i}")
                nc.sync.dma_start(out=t[:, :], in_=d_wo[i * 128:(i + 1) * 128, :])
                t_wo.append(t)
            t_qv = []
            for p in range(2):
                t = pp.tile([128, NQ], BF16, name=f"qv{p}", tag=f"qv{p}")
                nc.sync.dma_start(out=t[:, :], in_=d_qv[p, :, :])
                t_qv.append(t)

            # persistent per-pair stacked Q^T and final attn^T tiles
            # (final tiles split per q block so phase C of block n never
            # false-shares with phase B writes of block n+1)
            t_qt = [pp.tile([128, NQ], BF16, name=f"qt{p}", tag=f"qt{p}") for p in range(2)]
            t_f = {(p, qh): pp.tile([128, QB], BF16, name=f"f{p}q{qh}", tag=f"f{p}q{qh}")
                   for p in range(2) for qh in range(NQB)}

            # ---- Phase A: Q^T = Wq_pair^T @ x^T, stacked per head pair ----
            for p in range(2):
                for half in range(NQ // QB):
                    ps_q = scp.tile([128, QB], F32, name="ps", tag="sc")
                    for nb in range(QB // 512):
                        cs = slice(half * QB + nb * 512, half * QB + (nb + 1) * 512)
                        for mc in range(MC):
                            nc.tensor.matmul(
                                ps_q[:, nb * 512:(nb + 1) * 512],
                                t_wq[mc][:, p * 128:(p + 1) * 128],
                                t_xt[mc][:, cs],
                                start=(mc == 0), stop=(mc == MC - 1),
                            )
                    nc.vector.tensor_copy(t_qt[p][:, half * QB:(half + 1) * QB], ps_q[:, :])

            # ---- Phase B + C, software-pipelined across q blocks ----
            ATL = 2    # attn groups trail scores by 2 kt steps
            DEFER_KT = 5  # previous block's tail finish lands after this kt

            def attn_block(qh, p, deferred):
                """Scores + exp + attn for head pair (2p, 2p+1) on q block qh.
                `deferred` holds the previous block's tail-finish closures;
                they are emitted after kt=DEFER_KT so their PE ops (broadcast
                matmuls) never head-of-line-block this block's scores while
                waiting on the reciprocal chain. Returns this block's own
                tail-finish closures."""
                q0 = qh * QB
                at_ps = [atp.tile([128, QB], F32, name=f"at{hi}", tag=f"at{hi}")
                         for hi in range(2)]

                def at_mms(kt, probs):
                    for hi in range(2):
                        for nb in range(QB // 512):
                            s = slice(nb * 512, (nb + 1) * 512)
                            nc.tensor.matmul(
                                at_ps[hi][0:DK + 1, s],
                                t_vx[2 * p + hi][:, kt * (DK + 1):(kt + 1) * (DK + 1)],
                                probs[hi][:, s],
                                start=(kt == 0), stop=(kt == KT_TILES - 1),
                            )

                probs_hist = {}
                for kt in range(KT_TILES):
                    ks = slice(kt * 128, (kt + 1) * 128)
                    sc = []
                    for hi in range(2):
                        sc.append(scp.tile([128, QB], F32, name="sc", tag="sc"))
                    for nb in range(QB // 512):
                        s = slice(nb * 512, (nb + 1) * 512)
                        for hi, hr in ((0, slice(0, DK)), (1, slice(DK, 128))):
                            nc.tensor.matmul(
                                sc[hi][:, s], t_kt[p][hr, ks],
                                t_qt[p][hr, q0 + nb * 512:q0 + (nb + 1) * 512],
                                start=True, stop=True,
                            )
                    probs = []
                    for hi in range(2):
                        pt = prp.tile([128, QB], BF16, name=f"p{hi}", tag=f"p{hi}")
                        nc.scalar.activation(pt[:, :], sc[hi][:, :], AF.Exp)
                        probs.append(pt)
                    probs_hist[kt] = probs
                    if kt == DEFER_KT:
                        for fn in deferred:
                            fn()
                        deferred = []
                    if kt >= ATL:
                        at_mms(kt - ATL, probs_hist.pop(kt - ATL))
                for kt in range(KT_TILES - ATL, KT_TILES):
                    at_mms(kt, probs_hist.pop(kt))
                for fn in deferred:   # in case DEFER_KT was never reached
                    fn()

                # tail part 1 (now): copy attn+denominator off PSUM (frees the
                # at ring for the next block), then the reciprocal chain on
                # DVE + DMA, all off the PE critical path
                tail2 = []
                for hi in range(2):
                    t_at = sp.tile([DK + 1, QB], F32, name="at_sb", tag="at_sb")
                    nc.vector.tensor_copy(t_at[:, :], at_ps[hi][0:DK + 1, :])
                    t_dsc = sp.tile([128, QSC], F32, name="dsc", tag="dsc")
                    nc.sync.dma_start(out=t_dsc[:, :], in_=t_at[DK:DK + 1, :])
                    t_drc = sp.tile([128, QSC], F32, name="drc", tag="drc")
                    nc.vector.reciprocal(t_drc[:, :], t_dsc[:, :])
                    t_dra = sp.tile([128, QSC], F32, name="dra", tag="dra")
                    nc.vector.tensor_mul(t_dra[:, :], t_drc[:, :],
                                         t_qs[:, qh * QSC:(qh + 1) * QSC])
                    t_ra = sp.tile([1, QB], F32R, name="ra", tag="ra")
                    nc.sync.dma_start(out=t_ra[:, :], in_=t_dra[:, :].bitcast(F32R))

                    def fin(hi=hi, t_at=t_at, t_ra=t_ra):
                        ps_bc = scp.tile([128, QB], F32, name="bc", tag="sc")
                        for nb in range(QB // 512):
                            s = slice(nb * 512, (nb + 1) * 512)
                            nc.tensor.matmul(ps_bc[0:DK, s], t_on[:, :], t_ra[:, s],
                                             start=True, stop=True)
                        dst = t_f[(p, qh)][hi * DK:(hi + 1) * DK, :]
                        nc.vector.tensor_mul(dst, t_at[0:DK, :], ps_bc[0:DK, :])
                        nc.vector.tensor_add(dst, dst,
                                             t_qv[p][hi * DK:(hi + 1) * DK, q0:q0 + QB])
                    tail2.append(fin)
                return tail2

            def phase_c(qh):
                q0 = qh * QB
                for qt_i in range(QB // 128):
                    qs = slice(qt_i * 128, (qt_i + 1) * 128)
                    po = scp.tile([128, D], F32, name="po", tag="sc")
                    for nb in range(D // 512):
                        s = slice(nb * 512, (nb + 1) * 512)
                        nc.tensor.matmul(po[:, s], t_f[(0, qh)][:, qs], t_wo[0][:, s],
                                         start=True, stop=False)
                        nc.tensor.matmul(po[:, s], t_f[(1, qh)][:, qs], t_wo[1][:, s],
                                         start=False, stop=True)
                    t_out = op.tile([128, D], BF16, name="t_out", tag="t_out")
                    nc.vector.tensor_copy(t_out[:, :], po[:, :])
                    nc.sync.dma_start(out=d_out[q0 + qt_i * 128:q0 + (qt_i + 1) * 128, :],
                                      in_=t_out[:, :])

            # phase C of block qh is emitted after the first half of block
            # qh+1, so its matmuls never wait on the (qh, p=1) tail chain
            d = attn_block(0, 0, [])
            d = attn_block(0, 1, d)
            d = attn_block(1, 0, d)
            phase_c(0)
            d = attn_block(1, 1, d)
            for fn in d:
                fn()
            phase_c(1)

    _split_excess_waits(nc, limit=1)
    return nc


_PROGRAM = None


def _get_program():
    global _PROGRAM
    if _PROGRAM is None:
        _PROGRAM = _build_program()
    return _PROGRAM


def _core_inputs(c, x, K, V, Wq, Wo, kv_pad_mask, q_pad_mask):
    b = c // CPB
    g = c % CPB
    hs = slice(HPC * g, HPC * g + HPC)
    xt = np.ascontiguousarray(x[b].T).astype(BF)
    wq = np.ascontiguousarray(Wq[:, HPC * DK * g:HPC * DK * (g + 1)]).astype(BF)
    kth = K[b, hs].transpose(0, 2, 1) * np.float32(SCALE)     # [4, DK, NKV]
    kt2 = np.ascontiguousarray(kth.reshape(2, 128, NKV)).astype(BF)
    kvm = kv_pad_mask[b, 0, 0].astype(bool)
    # kv mask folded into V_ext: zeroed rows (incl. the ones column) drop
    # masked kv from both the attn accumulation and the denominator, so
    # exp needs no mask bias at all
    v = V[b, hs] * kvm[None, :, None]
    v = v.reshape(HPC, KT_TILES, 128, DK).transpose(0, 2, 1, 3)
    ones = np.broadcast_to(
        kvm.reshape(KT_TILES, 128).T.astype(np.float32)[None, :, :, None],
        (HPC, 128, KT_TILES, 1))
    vext = np.concatenate([v, ones], axis=-1).reshape(
        HPC, 128, KT_TILES * (DK + 1)).astype(BF)
    wo = np.ascontiguousarray(Wo[HPC * DK * g:HPC * DK * (g + 1), :]).astype(BF)
    qm = q_pad_mask[b, 0, :, 0].astype(np.float32)            # [NQ]
    QSC = QB // 128
    qmsc = np.ascontiguousarray(
        qm.reshape(NQB, 128, QSC).transpose(1, 0, 2).reshape(128, NQB * QSC))
    mv = V[b, hs].mean(axis=1, dtype=np.float32)              # [4, DK]
    qbmv = (mv[:, :, None] * (1.0 - qm)[None, None, :]).reshape(2, 128, NQ).astype(BF)
    return dict(xt=xt, wq=wq, kt2=kt2, vext=vext, wo=wo,
                qmsc=qmsc, qbmv=qbmv, ones=np.ones((1, DK), np.float32))


def _install_ntff_hook():
    """The axon NTFF profile hook normally lives in antenv.axon_hooks,
    which this image lacks. Recreate it from trn_agent_boot so
    trace=True profiling works."""
    import sys
    import types
    try:
        from antenv.axon_hooks import get_axon_ntff_profile_hook  # noqa: F401
        return
    except ImportError:
        pass
    try:
        from trn_agent_boot.trn_boot import _ntff_profile_via_ctypes
        hook = _ntff_profile_via_ctypes("/opt/axon/libaxon_pjrt.so")
    except Exception:
        hook = None
    m = types.ModuleType("antenv.axon_hooks")
    m.get_axon_ntff_profile_hook = lambda: hook
    m.set_axon_ntff_profile_hook = lambda h: None
    sys.modules["antenv.axon_hooks"] = m


def kernel(x, K, V, Wq, Wo, kv_pad_mask, q_pad_mask, _trace=False):
    if _trace:
        _install_ntff_hook()
    nc = _get_program()
    x = np.asarray(x)
    K = np.asarray(K)
    V = np.asarray(V)
    Wq = np.asarray(Wq)
    Wo = np.asarray(Wo)
    kv_pad_mask = np.asarray(kv_pad_mask)
    q_pad_mask = np.asarray(q_pad_mask)
    in_maps = [_core_inputs(c, x, K, V, Wq, Wo, kv_pad_mask, q_pad_mask)
               for c in range(B * CPB)]
    res = run_bass_kernel_spmd(nc, in_maps, list(range(B * CPB)), trace=_trace)
    kernel._last_exec_ns = res.exec_time_ns
    kernel._last_results = res
    out = np.empty((B, NQ, D), np.float32)
    for b in range(B):
        acc = res.results[b * CPB]["out"].astype(np.float32)
        for j in range(1, CPB):
            acc = acc + res.results[b * CPB + j]["out"].astype(np.float32)
        out[b] = acc
    return out


kernel._last_exec_ns = None
kernel._last_results = None
